# revision 4
# baseline (speedup 1.0000x reference)
"""Trainium2 Bass kernel for nn_CLIPCrossProductClassifier.

Math:  y[b,h] = sum_{i,j} img_n[b,i] * txt_n[b,j] * W1r[i,j,h]
       logits = relu(y + b1) @ W2 + b2
with img_n/txt_n the L2-normalized embeddings and W1r = W1.reshape(D,D,H).

Sharding: contraction-parallel over i (rows of the bilinear form). Each of
the 8 cores owns 64 values of i (a [64*D, H] row-slice of W1, 32 MB in fp16)
and computes a partial y_c[b,h]. Partials are summed on the host (8 x 1 MB),
followed by the tiny bias/ReLU/[512x1] projection.

v2 schedule ("all-PSUM"): the per-i img scale is folded into the matmul
stationary operand instead of being applied to the matmul output:
  imgb[i]      = broadcast img[:, i] across 128 partitions     (GPSIMD)
  scaled[i][c] = txtT[c] * imgb[i]                             (DVE, fp16)
  ps[bb]      += scaled[i][c][:, bb].T @ W1[i][c]              (PE, fp16)
so each of the 4 batch-block PSUM banks accumulates all 256 matmuls
(64 i x 4 j-chunks) with start/stop only at the ends. This removes the
v1 per-i epilogue (ACT scale + DVE add over [B,H], ~200us/engine) and its
PSUM-recycling backpressure on the PE; the PE's only dependencies are the
DVE-produced stationaries (~2.2x headroom) and the W1 slab DMA stream
(fp16 halves it to ~140 GB/s demand vs ~300+ available).
W1 is host-transposed per i to [j_part, chunk, h] so each i is one
contiguous 512 KB DMA. i=0 is split into 4 chunk tiles for a fast start.
The last iteration runs bb-major so each bank's drain (ACT copy + DMA)
hides under the next bank's matmuls.
"""

import numpy as np

import concourse.bass as bass
import concourse.tile as tile
from concourse import bacc, mybir
from concourse.bass_utils import run_bass_kernel_spmd

B, D, H = 512, 512, 512
N_CORES = 8
I_PER_CORE = D // N_CORES          # 64
N_BBLK = B // 128                  # 4
N_JCHUNK = D // 128                # 4
EPS = 1e-12

F32 = mybir.dt.float32
F32R = mybir.dt.float32r
F16 = mybir.dt.float16

MM_MODE = "v2"
# v1 modes -> (txt/stationary dtype, W1/moving dtype, numpy dtypes for each).
_MM_DT = {
    "f32r": (F32R, F32R, np.float32, np.float32),
    "f16": (F16, F16, np.float16, np.float16),
}

_CACHE = {}


def _l2norm(x: np.ndarray) -> np.ndarray:
    n = np.sqrt(np.sum(x * x, axis=1, keepdims=True, dtype=np.float32))
    return (x / np.maximum(n, np.float32(EPS))).astype(np.float32)


def build_nc_v2():
    """Per-core Bass program, v2 all-PSUM schedule (SPMD, per-core data)."""
    nc = bacc.Bacc(
        "TRN2",
        target_bir_lowering=False,
        debug=False,
        num_devices=N_CORES,
    )

    txt_t = nc.dram_tensor("txt_t", [D, B], F16, kind="ExternalInput").ap()
    img_t = nc.dram_tensor("img_t", [I_PER_CORE, B], F16, kind="ExternalInput").ap()
    # slab[i, p, c*H + h] = W1r[i_global, c*128 + p, h]: one contiguous 512 KB
    # transfer per i, partitions = j within chunk.
    w1_s = nc.dram_tensor(
        "w1_s", [I_PER_CORE, 128, N_JCHUNK * H], F16, kind="ExternalInput"
    ).ap()
    yp = nc.dram_tensor("yp", [B, H], F32, kind="ExternalOutput").ap()

    with tile.TileContext(nc) as tc:
        with (
            tc.tile_pool(name="const", bufs=1) as constp,
            tc.tile_pool(name="w1f", bufs=1) as w1fp,
            tc.tile_pool(name="w1", bufs=10) as w1p,
            tc.tile_pool(name="imgb", bufs=3) as imgbp,
            tc.tile_pool(name="scl", bufs=3) as sclp,
            tc.tile_pool(name="out", bufs=1) as outp,
            tc.tile_pool(name="ps", bufs=1, space=bass.MemorySpace.PSUM) as psump,
        ):
            # img lives on partition 0 along the free axis: partition_broadcast
            # requires its input to start at partition 0, so img_f[0:1, i*B:]
            # keeps every per-i slice legal. Row 0 is loaded separately so the
            # i=0 chain doesn't wait for the whole 64 KB single-engine DMA.
            img_f = constp.tile([1, I_PER_CORE * B], F16, tag="img", name="img_f")
            nc.sync.dma_start(img_f[0:1, 0:B], img_t[0:1, :])
            nc.sync.dma_start(img_f[0:1, B:], img_t[1:, :])
            txt_sb = []
            for c in range(N_JCHUNK):
                t = constp.tile([128, B], F16, tag=f"txt{c}", name=f"txt_sb{c}")
                nc.sync.dma_start(t[:], txt_t[c * 128 : (c + 1) * 128, :])
                txt_sb.append(t)
            # i=0 W1 as 4 chunk tiles so the first matmul waits on 128 KB,
            # not the full 512 KB slab.
            w1t0 = [
                w1fp.tile([128, H], F16, tag=f"w1f{c}", name=f"w1t0c{c}")
                for c in range(N_JCHUNK)
            ]
            for c in range(N_JCHUNK):
                nc.sync.dma_start(w1t0[c][:], w1_s[0, :, c * H : (c + 1) * H])

            ps = [
                psump.tile([128, H], F32, tag=f"ps{bb}", name=f"ps{bb}")
                for bb in range(N_BBLK)
            ]
            acc_sb = [
                outp.tile([128, H], F32, tag=f"o{bb}", name=f"acc_sb{bb}")
                for bb in range(N_BBLK)
            ]

            for i in range(I_PER_CORE):
                if i == 0:
                    w1v = [w1t0[c][:] for c in range(N_JCHUNK)]
                else:
                    slab = w1p.tile(
                        [128, N_JCHUNK * H], F16, tag="w1", name="w1slab"
                    )
                    nc.sync.dma_start(slab[:], w1_s[i])
                    w1v = [slab[:, c * H : (c + 1) * H] for c in range(N_JCHUNK)]

                imgb = imgbp.tile([128, B], F16, tag="imgb", name="imgb")
                nc.gpsimd.partition_broadcast(
                    imgb[:], img_f[0:1, i * B : (i + 1) * B], channels=128
                )
                scl = []
                for c in range(N_JCHUNK):
                    s = sclp.tile([128, B], F16, tag=f"s{c}", name=f"scl{c}")
                    nc.vector.tensor_tensor(
                        s[:], txt_sb[c][:], imgb[:], mybir.AluOpType.mult
                    )
                    scl.append(s)

                if i < I_PER_CORE - 1:
                    for c in range(N_JCHUNK):
                        for bb in range(N_BBLK):
                            nc.tensor.matmul(
                                ps[bb][:],
                                scl[c][:, bb * 128 : (bb + 1) * 128],
                                w1v[c],
                                start=(i == 0 and c == 0),
                                stop=False,
                                skip_group_check=not (i == 0 and c == 0),
                            )
                else:
                    # Last i: bb-major so bank bb's drain overlaps bank bb+1's
                    # matmuls.
                    for bb in range(N_BBLK):
                        for c in range(N_JCHUNK):
                            nc.tensor.matmul(
                                ps[bb][:],
                                scl[c][:, bb * 128 : (bb + 1) * 128],
                                w1v[c],
                                start=False,
                                stop=(c == N_JCHUNK - 1),
                                skip_group_check=(c != N_JCHUNK - 1),
                            )
                        nc.scalar.activation(
                            acc_sb[bb][:],
                            ps[bb][:],
                            mybir.ActivationFunctionType.Copy,
                        )
                        nc.sync.dma_start(
                            yp[bb * 128 : (bb + 1) * 128, :], acc_sb[bb][:]
                        )

    nc.compile()
    return nc


def make_in_maps_v2(image_embeds, text_embeds, W1):
    imgn = _l2norm(np.asarray(image_embeds, np.float32))
    txtn = _l2norm(np.asarray(text_embeds, np.float32))
    txt_t = np.ascontiguousarray(txtn.T).astype(np.float16)
    W1r = np.asarray(W1, np.float32).reshape(D, D, H)
    in_maps = []
    for c in range(N_CORES):
        sl = W1r[c * I_PER_CORE : (c + 1) * I_PER_CORE]  # [64, 512, 512]
        slab = np.ascontiguousarray(
            sl.reshape(I_PER_CORE, N_JCHUNK, 128, H).transpose(0, 2, 1, 3)
        ).reshape(I_PER_CORE, 128, N_JCHUNK * H).astype(np.float16)
        img_t = np.ascontiguousarray(
            imgn[:, c * I_PER_CORE : (c + 1) * I_PER_CORE].T
        ).astype(np.float16)
        in_maps.append({"txt_t": txt_t, "img_t": img_t, "w1_s": slab})
    return in_maps


# ---------------------------------------------------------------------------
# v1 path (per-i PSUM drain + ACT/DVE epilogue), kept for A/B comparisons.
# ---------------------------------------------------------------------------


def build_nc_v1(mm):
    txt_dt, w1_dt = _MM_DT[mm][0], _MM_DT[mm][1]
    nc = bacc.Bacc(
        "TRN2",
        target_bir_lowering=False,
        debug=False,
        num_devices=N_CORES,
    )

    txt_t = nc.dram_tensor("txt_t", [D, B], txt_dt, kind="ExternalInput").ap()
    img_s = nc.dram_tensor("img_s", [B, I_PER_CORE], F32, kind="ExternalInput").ap()
    w1_s = nc.dram_tensor(
        "w1_s", [I_PER_CORE, N_JCHUNK, 128, H], w1_dt, kind="ExternalInput"
    ).ap()
    yp = nc.dram_tensor("yp", [B, H], F32, kind="ExternalOutput").ap()

    with tile.TileContext(nc) as tc:
        with (
            tc.tile_pool(name="const", bufs=1) as constp,
            tc.tile_pool(name="w1", bufs=6) as w1p,
            tc.tile_pool(name="accs", bufs=1) as accp,
            tc.tile_pool(name="scl", bufs=6) as sclp,
            tc.tile_pool(name="ps", bufs=6, space=bass.MemorySpace.PSUM) as psump,
        ):
            w1t0 = [
                w1p.tile([128, H], w1_dt, tag=f"w1c{c}", name=f"w1c{c}p")
                for c in range(N_JCHUNK)
            ]
            for c in range(N_JCHUNK):
                nc.sync.dma_start(w1t0[c][:], w1_s[0, c])

            txt_sb = []
            for c in range(N_JCHUNK):
                halves = []
                for hh in range(2):
                    t = constp.tile(
                        [128, B // 2], txt_dt,
                        tag=f"txt{c}h{hh}", name=f"txt_sb{c}h{hh}",
                    )
                    nc.sync.dma_start(
                        t[:],
                        txt_t[c * 128 : (c + 1) * 128,
                              hh * (B // 2) : (hh + 1) * (B // 2)],
                    )
                    halves.append(t)
                txt_sb.append(halves)
            img_sb = []
            for bb in range(N_BBLK):
                t = constp.tile([128, I_PER_CORE], F32, tag=f"img{bb}", name=f"img_sb{bb}")
                nc.sync.dma_start(t[:], img_s[bb * 128 : (bb + 1) * 128, :])
                img_sb.append(t)
            acc = [
                accp.tile([128, H], F32, tag=f"acc{bb}", name=f"acc{bb}")
                for bb in range(N_BBLK)
            ]

            for i in range(I_PER_CORE):
                if i == 0:
                    w1t = w1t0
                else:
                    w1t = [
                        w1p.tile([128, H], w1_dt, tag=f"w1c{c}", name=f"w1c{c}")
                        for c in range(N_JCHUNK)
                    ]
                    for c in range(N_JCHUNK):
                        nc.sync.dma_start(w1t[c][:], w1_s[i, c])
                for bb in range(N_BBLK):
                    ps = psump.tile([128, H], F32, tag="ps")
                    for c in range(N_JCHUNK):
                        lhs = txt_sb[c][bb // 2]
                        col = (bb % 2) * 128
                        nc.tensor.matmul(
                            ps[:],
                            lhs[:, col : col + 128],
                            w1t[c][:],
                            start=(c == 0),
                            stop=(c == N_JCHUNK - 1),
                        )
                    sc = img_sb[bb][:, i : i + 1]
                    if i == 0:
                        nc.scalar.activation(
                            acc[bb][:], ps[:], mybir.ActivationFunctionType.Copy,
                            scale=sc,
                        )
                    else:
                        scaled = sclp.tile([128, H], F32, tag="scaled", name="scaled")
                        nc.scalar.activation(
                            scaled[:], ps[:], mybir.ActivationFunctionType.Copy,
                            scale=sc,
                        )
                        nc.vector.tensor_add(acc[bb][:], acc[bb][:], scaled[:])

            for bb in range(N_BBLK):
                nc.sync.dma_start(yp[bb * 128 : (bb + 1) * 128, :], acc[bb][:])

    nc.compile()
    return nc


def make_in_maps_v1(image_embeds, text_embeds, W1, mm):
    txt_np, w1_np = _MM_DT[mm][2], _MM_DT[mm][3]
    imgn = _l2norm(np.asarray(image_embeds, np.float32))
    txtn = _l2norm(np.asarray(text_embeds, np.float32))
    txt_t = np.ascontiguousarray(txtn.T).astype(txt_np)
    W1r = np.asarray(W1, np.float32).reshape(D, D, H).astype(w1_np)
    in_maps = []
    for c in range(N_CORES):
        w1c = W1r[c * I_PER_CORE : (c + 1) * I_PER_CORE].reshape(
            I_PER_CORE, N_JCHUNK, 128, H
        )
        in_maps.append(
            {
                "txt_t": txt_t,
                "img_s": np.ascontiguousarray(imgn[:, c * I_PER_CORE : (c + 1) * I_PER_CORE]),
                "w1_s": w1c,
            }
        )
    return in_maps


def make_in_maps(image_embeds, text_embeds, W1, mm=MM_MODE):
    if mm == "v2":
        return make_in_maps_v2(image_embeds, text_embeds, W1)
    return make_in_maps_v1(image_embeds, text_embeds, W1, mm)


def run_device(in_maps, trace=False, mm=MM_MODE, **kw):
    if mm not in _CACHE:
        _CACHE[mm] = build_nc_v2() if mm == "v2" else build_nc_v1(mm)
    return run_bass_kernel_spmd(
        _CACHE[mm], in_maps, list(range(N_CORES)), trace=trace, **kw
    )


def finish_host(results, b1, W2, b2):
    Y = np.zeros((B, H), np.float32)
    for c in range(N_CORES):
        Y += results[c]["yp"]
    h = np.maximum(Y + np.asarray(b1, np.float32), np.float32(0.0))
    out = h @ np.asarray(W2, np.float32) + np.asarray(b2, np.float32)
    return out.astype(np.float32)


def kernel(image_embeds, text_embeds, W1, b1, W2, b2):
    in_maps = make_in_maps(image_embeds, text_embeds, W1)
    res = run_device(in_maps, trace=False)
    return finish_host(res.results, b1, W2, b2)


# revision 5
# speedup vs baseline: 1.1787x; 1.1787x over previous
"""Trainium2 Bass kernel for nn_CLIPCrossProductClassifier.

Math:  y[b,h] = sum_{i,j} img_n[b,i] * txt_n[b,j] * W1r[i,j,h]
       logits = relu(y + b1) @ W2 + b2
with img_n/txt_n the L2-normalized embeddings and W1r = W1.reshape(D,D,H).

Sharding: contraction-parallel over i (rows of the bilinear form). Each of
the 8 cores owns 64 values of i (a [64*D, H] row-slice of W1, 32 MB in fp16)
and computes a partial y_c[b,h]. Partials are summed on the host (8 x 1 MB),
followed by the tiny bias/ReLU/[512x1] projection.

v2 schedule ("all-PSUM"): the per-i img scale is folded into the matmul
stationary operand instead of being applied to the matmul output:
  imgb[i]      = broadcast img[:, i] across 128 partitions     (GPSIMD)
  scaled[i][c] = txtT[c] * imgb[i]                             (DVE, fp16)
  ps[bb]      += scaled[i][c][:, bb].T @ W1[i][c]              (PE, fp16)
so each of the 4 batch-block PSUM banks accumulates all 256 matmuls
(64 i x 4 j-chunks) with start/stop only at the ends. This removes the
v1 per-i epilogue (ACT scale + DVE add over [B,H], ~200us/engine) and its
PSUM-recycling backpressure on the PE. All 64 partition_broadcasts are
emitted up front: GPSIMD (1.2us each) runs ~3x ahead of PE consumption
(3.6us/i), so the per-i critical chain is only the 4 DVE multiplies.
img lives on partition 0 along the free axis because partition_broadcast
input must start at partition 0. The last iteration runs bb-major so each
bank's drain (ACT copy + DMA) hides under the next bank's matmuls.
"""

import numpy as np

import concourse.bass as bass
import concourse.tile as tile
from concourse import bacc, mybir
from concourse.bass_utils import run_bass_kernel_spmd

B, D, H = 512, 512, 512
N_CORES = 8
I_PER_CORE = D // N_CORES          # 64
N_BBLK = B // 128                  # 4
N_JCHUNK = D // 128                # 4
EPS = 1e-12

F32 = mybir.dt.float32
F32R = mybir.dt.float32r
F16 = mybir.dt.float16

MM_MODE = "v2"
# v1 modes -> (txt/stationary dtype, W1/moving dtype, numpy dtypes for each).
_MM_DT = {
    "f32r": (F32R, F32R, np.float32, np.float32),
    "f16": (F16, F16, np.float16, np.float16),
}

_CACHE = {}


def _l2norm(x: np.ndarray) -> np.ndarray:
    n = np.sqrt(np.sum(x * x, axis=1, keepdims=True, dtype=np.float32))
    return (x / np.maximum(n, np.float32(EPS))).astype(np.float32)


def build_nc_v2():
    """Per-core Bass program, v2 all-PSUM schedule (SPMD, per-core data)."""
    nc = bacc.Bacc(
        "TRN2",
        target_bir_lowering=False,
        debug=False,
        num_devices=N_CORES,
    )

    txt_t = nc.dram_tensor("txt_t", [D, B], F16, kind="ExternalInput").ap()
    img_t = nc.dram_tensor("img_t", [I_PER_CORE, B], F16, kind="ExternalInput").ap()
    w1_s = nc.dram_tensor(
        "w1_s", [I_PER_CORE, N_JCHUNK, 128, H], F16, kind="ExternalInput"
    ).ap()
    yp = nc.dram_tensor("yp", [B, H], F32, kind="ExternalOutput").ap()

    with tile.TileContext(nc) as tc:
        with (
            tc.tile_pool(name="const", bufs=1) as constp,
            tc.tile_pool(name="w1", bufs=8) as w1p,
            tc.tile_pool(name="imgb", bufs=32) as imgbp,
            tc.tile_pool(name="scl", bufs=4) as sclp,
            tc.tile_pool(name="out", bufs=1) as outp,
            tc.tile_pool(name="ps", bufs=1, space=bass.MemorySpace.PSUM) as psump,
        ):
            # img on partition 0 along the free axis (partition_broadcast needs
            # partition-0 input). Row 0 lands first so the i=0 chain starts fast.
            img_f = constp.tile([1, I_PER_CORE * B], F16, tag="img", name="img_f")
            nc.sync.dma_start(img_f[0:1, 0:B], img_t[0:1, :])
            nc.sync.dma_start(img_f[0:1, B:], img_t[1:, :])
            txt_sb = []
            for c in range(N_JCHUNK):
                t = constp.tile([128, B], F16, tag=f"txt{c}", name=f"txt_sb{c}")
                nc.sync.dma_start(t[:], txt_t[c * 128 : (c + 1) * 128, :])
                txt_sb.append(t)
            # Prefetch the first two i of W1 ahead of the loop.
            w1pre = {}
            for i in range(2):
                for c in range(N_JCHUNK):
                    t = w1p.tile([128, H], F16, tag=f"w1c{c}", name=f"w1c{c}p{i}")
                    nc.sync.dma_start(t[:], w1_s[i, c])
                    w1pre[(i, c)] = t

            # All 64 partition broadcasts up front; GPSIMD runs ahead, the
            # 32-deep ring gives ~32 iterations of lookahead.
            imgb = []
            for i in range(I_PER_CORE):
                t = imgbp.tile([128, B], F16, tag="imgb", name=f"imgb{i}")
                nc.gpsimd.partition_broadcast(
                    t[:], img_f[0:1, i * B : (i + 1) * B], channels=128
                )
                imgb.append(t)

            ps = [
                psump.tile([128, H], F32, tag=f"ps{bb}", name=f"ps{bb}")
                for bb in range(N_BBLK)
            ]
            acc_sb = [
                outp.tile([128, H], F32, tag=f"o{bb}", name=f"acc_sb{bb}")
                for bb in range(N_BBLK)
            ]

            for i in range(I_PER_CORE):
                if i < 2:
                    w1t = [w1pre[(i, c)] for c in range(N_JCHUNK)]
                else:
                    w1t = [
                        w1p.tile([128, H], F16, tag=f"w1c{c}", name=f"w1c{c}")
                        for c in range(N_JCHUNK)
                    ]
                    for c in range(N_JCHUNK):
                        nc.sync.dma_start(w1t[c][:], w1_s[i, c])

                scl = []
                for c in range(N_JCHUNK):
                    s = sclp.tile([128, B], F16, tag=f"s{c}", name=f"scl{c}")
                    nc.vector.tensor_tensor(
                        s[:], txt_sb[c][:], imgb[i][:], mybir.AluOpType.mult
                    )
                    scl.append(s)

                if i < I_PER_CORE - 1:
                    for c in range(N_JCHUNK):
                        for bb in range(N_BBLK):
                            nc.tensor.matmul(
                                ps[bb][:],
                                scl[c][:, bb * 128 : (bb + 1) * 128],
                                w1t[c][:],
                                start=(i == 0 and c == 0),
                                stop=False,
                                skip_group_check=not (i == 0 and c == 0),
                            )
                else:
                    # Last i: bb-major so bank bb's drain overlaps bank bb+1's
                    # matmuls.
                    for bb in range(N_BBLK):
                        for c in range(N_JCHUNK):
                            nc.tensor.matmul(
                                ps[bb][:],
                                scl[c][:, bb * 128 : (bb + 1) * 128],
                                w1t[c][:],
                                start=False,
                                stop=(c == N_JCHUNK - 1),
                                skip_group_check=(c != N_JCHUNK - 1),
                            )
                        nc.scalar.activation(
                            acc_sb[bb][:],
                            ps[bb][:],
                            mybir.ActivationFunctionType.Copy,
                        )
                        nc.sync.dma_start(
                            yp[bb * 128 : (bb + 1) * 128, :], acc_sb[bb][:]
                        )

    nc.compile()
    return nc


def make_in_maps_v2(image_embeds, text_embeds, W1):
    imgn = _l2norm(np.asarray(image_embeds, np.float32))
    txtn = _l2norm(np.asarray(text_embeds, np.float32))
    txt_t = np.ascontiguousarray(txtn.T).astype(np.float16)
    W1r = np.asarray(W1, np.float32).reshape(D, D, H)
    in_maps = []
    for c in range(N_CORES):
        w1c = (
            W1r[c * I_PER_CORE : (c + 1) * I_PER_CORE]
            .reshape(I_PER_CORE, N_JCHUNK, 128, H)
            .astype(np.float16)
        )
        img_t = np.ascontiguousarray(
            imgn[:, c * I_PER_CORE : (c + 1) * I_PER_CORE].T
        ).astype(np.float16)
        in_maps.append({"txt_t": txt_t, "img_t": img_t, "w1_s": w1c})
    return in_maps


# ---------------------------------------------------------------------------
# v1 path (per-i PSUM drain + ACT/DVE epilogue), kept for A/B comparisons.
# ---------------------------------------------------------------------------


def build_nc_v1(mm):
    txt_dt, w1_dt = _MM_DT[mm][0], _MM_DT[mm][1]
    nc = bacc.Bacc(
        "TRN2",
        target_bir_lowering=False,
        debug=False,
        num_devices=N_CORES,
    )

    txt_t = nc.dram_tensor("txt_t", [D, B], txt_dt, kind="ExternalInput").ap()
    img_s = nc.dram_tensor("img_s", [B, I_PER_CORE], F32, kind="ExternalInput").ap()
    w1_s = nc.dram_tensor(
        "w1_s", [I_PER_CORE, N_JCHUNK, 128, H], w1_dt, kind="ExternalInput"
    ).ap()
    yp = nc.dram_tensor("yp", [B, H], F32, kind="ExternalOutput").ap()

    with tile.TileContext(nc) as tc:
        with (
            tc.tile_pool(name="const", bufs=1) as constp,
            tc.tile_pool(name="w1", bufs=6) as w1p,
            tc.tile_pool(name="accs", bufs=1) as accp,
            tc.tile_pool(name="scl", bufs=6) as sclp,
            tc.tile_pool(name="ps", bufs=6, space=bass.MemorySpace.PSUM) as psump,
        ):
            w1t0 = [
                w1p.tile([128, H], w1_dt, tag=f"w1c{c}", name=f"w1c{c}p")
                for c in range(N_JCHUNK)
            ]
            for c in range(N_JCHUNK):
                nc.sync.dma_start(w1t0[c][:], w1_s[0, c])

            txt_sb = []
            for c in range(N_JCHUNK):
                halves = []
                for hh in range(2):
                    t = constp.tile(
                        [128, B // 2], txt_dt,
                        tag=f"txt{c}h{hh}", name=f"txt_sb{c}h{hh}",
                    )
                    nc.sync.dma_start(
                        t[:],
                        txt_t[c * 128 : (c + 1) * 128,
                              hh * (B // 2) : (hh + 1) * (B // 2)],
                    )
                    halves.append(t)
                txt_sb.append(halves)
            img_sb = []
            for bb in range(N_BBLK):
                t = constp.tile([128, I_PER_CORE], F32, tag=f"img{bb}", name=f"img_sb{bb}")
                nc.sync.dma_start(t[:], img_s[bb * 128 : (bb + 1) * 128, :])
                img_sb.append(t)
            acc = [
                accp.tile([128, H], F32, tag=f"acc{bb}", name=f"acc{bb}")
                for bb in range(N_BBLK)
            ]

            for i in range(I_PER_CORE):
                if i == 0:
                    w1t = w1t0
                else:
                    w1t = [
                        w1p.tile([128, H], w1_dt, tag=f"w1c{c}", name=f"w1c{c}")
                        for c in range(N_JCHUNK)
                    ]
                    for c in range(N_JCHUNK):
                        nc.sync.dma_start(w1t[c][:], w1_s[i, c])
                for bb in range(N_BBLK):
                    ps = psump.tile([128, H], F32, tag="ps")
                    for c in range(N_JCHUNK):
                        lhs = txt_sb[c][bb // 2]
                        col = (bb % 2) * 128
                        nc.tensor.matmul(
                            ps[:],
                            lhs[:, col : col + 128],
                            w1t[c][:],
                            start=(c == 0),
                            stop=(c == N_JCHUNK - 1),
                        )
                    sc = img_sb[bb][:, i : i + 1]
                    if i == 0:
                        nc.scalar.activation(
                            acc[bb][:], ps[:], mybir.ActivationFunctionType.Copy,
                            scale=sc,
                        )
                    else:
                        scaled = sclp.tile([128, H], F32, tag="scaled", name="scaled")
                        nc.scalar.activation(
                            scaled[:], ps[:], mybir.ActivationFunctionType.Copy,
                            scale=sc,
                        )
                        nc.vector.tensor_add(acc[bb][:], acc[bb][:], scaled[:])

            for bb in range(N_BBLK):
                nc.sync.dma_start(yp[bb * 128 : (bb + 1) * 128, :], acc[bb][:])

    nc.compile()
    return nc


def make_in_maps_v1(image_embeds, text_embeds, W1, mm):
    txt_np, w1_np = _MM_DT[mm][2], _MM_DT[mm][3]
    imgn = _l2norm(np.asarray(image_embeds, np.float32))
    txtn = _l2norm(np.asarray(text_embeds, np.float32))
    txt_t = np.ascontiguousarray(txtn.T).astype(txt_np)
    W1r = np.asarray(W1, np.float32).reshape(D, D, H).astype(w1_np)
    in_maps = []
    for c in range(N_CORES):
        w1c = W1r[c * I_PER_CORE : (c + 1) * I_PER_CORE].reshape(
            I_PER_CORE, N_JCHUNK, 128, H
        )
        in_maps.append(
            {
                "txt_t": txt_t,
                "img_s": np.ascontiguousarray(imgn[:, c * I_PER_CORE : (c + 1) * I_PER_CORE]),
                "w1_s": w1c,
            }
        )
    return in_maps


def make_in_maps(image_embeds, text_embeds, W1, mm=MM_MODE):
    if mm == "v2":
        return make_in_maps_v2(image_embeds, text_embeds, W1)
    return make_in_maps_v1(image_embeds, text_embeds, W1, mm)


def run_device(in_maps, trace=False, mm=MM_MODE, **kw):
    if mm not in _CACHE:
        _CACHE[mm] = build_nc_v2() if mm == "v2" else build_nc_v1(mm)
    return run_bass_kernel_spmd(
        _CACHE[mm], in_maps, list(range(N_CORES)), trace=trace, **kw
    )


def finish_host(results, b1, W2, b2):
    Y = np.zeros((B, H), np.float32)
    for c in range(N_CORES):
        Y += results[c]["yp"]
    h = np.maximum(Y + np.asarray(b1, np.float32), np.float32(0.0))
    out = h @ np.asarray(W2, np.float32) + np.asarray(b2, np.float32)
    return out.astype(np.float32)


def kernel(image_embeds, text_embeds, W1, b1, W2, b2):
    in_maps = make_in_maps(image_embeds, text_embeds, W1)
    res = run_device(in_maps, trace=False)
    return finish_host(res.results, b1, W2, b2)
